# revision 1
# baseline (speedup 1.0000x reference)
"""MeshGraphDecoder Trainium2 kernel (8-core SPMD).

Sharding: grid nodes (and their incoming edges) are partitioned across 8
cores; mesh features and MLP weights are replicated (gathered on host
into per-edge streams). Within a core, nodes are packed into 256
windows of 128 nodes such that each window's incoming-edge count fits a
global per-window capacity schedule (T_w*128 slots, T_w in {3,4}); the
schedule is shared by all cores so one SPMD program serves all 8.

Device pipeline per window w (ET = 128*T_w edges):
  edge MLP   : catT [384, ET] chunks -> 6 matmuls -> H [256, ET] PSUM
               -> SiLU+b1 (ACT) -> per-128-edge tile: 2 matmuls ->
               ef2 [128e,128d] -> +b2, LayerNorm (DVE)
  aggregate  : onehotT[e,n] = (slot[e] == iota[n]) ; aggT [128d,128n]
               += ef2^T @ onehotT  (PSUM-accumulated over T_w tiles)
  node MLP   : per 4 windows (512 nodes): cat(aggT, gridT) -> 4 matmuls
               -> SiLU+b1 -> per-128-node tile: 2 matmuls -> +b2,
               LayerNorm, +grid residual -> out rows
"""

import numpy as np

N_MESH = 40962
N_GRID = 262144
N_EDGE = 786432
D = 128
HID = 256
EPS = 1e-5
N_CORES = 8
N_SH = N_GRID // N_CORES
W_PER_CORE = N_SH // 128
P = 128
SW = 4  # windows per node-stage supertile


# ----------------------------------------------------------------- host prep

def _pack_core(degrees, caps):
    n = len(degrees)
    n_win = len(caps)
    caps = np.asarray(caps, dtype=np.int64)
    order = np.argsort(-degrees, kind="stable")
    sums = np.zeros(n_win, dtype=np.int64)
    cnts = np.zeros(n_win, dtype=np.int64)
    assign = np.empty(n, dtype=np.int64)
    mean = degrees.sum() / n
    caps_f = caps.astype(np.float64)
    for nid in order:
        slack = (caps_f - sums) - mean * (128 - cnts)
        slack[cnts >= 128] = -np.inf
        w = int(np.argmax(slack))
        assign[nid] = w
        sums[w] += degrees[nid]
        cnts[w] += 1
    members = [list(np.nonzero(assign == w)[0]) for w in range(n_win)]
    for _ in range(200000):
        over = np.nonzero(sums > caps)[0]
        if len(over) == 0:
            break
        w = int(over[0])
        mw = members[w]
        a = max(mw, key=lambda i: degrees[i])
        v = int(np.argmax(caps - sums))
        mv = members[v]
        b = min(mv, key=lambda i: degrees[i])
        da, db = int(degrees[a]), int(degrees[b])
        assert da > db and (caps[v] - sums[v]) >= (da - db), "repair stuck"
        mw.remove(a); mv.remove(b)
        mw.append(b); mv.append(a)
        sums[w] += db - da
        sums[v] += da - db
    else:
        raise RuntimeError("window repair did not converge")
    perm = np.empty(n, dtype=np.int64)
    for w in range(n_win):
        perm[w * 128 : (w + 1) * 128] = members[w]
    return perm


def _prepare(inputs):
    dst = np.asarray(inputs["dst_idx"]).astype(np.int64)
    src = np.asarray(inputs["src_idx"]).astype(np.int64)
    ef = np.asarray(inputs["m2g_efeat"], dtype=np.float32)
    gf = np.asarray(inputs["grid_nfeat"], dtype=np.float32)
    mf = np.asarray(inputs["mesh_nfeat"], dtype=np.float32)

    core_of_edge = dst // N_SH
    e_counts = np.bincount(core_of_edge, minlength=N_CORES)
    base = W_PER_CORE * 3 * 128
    a = max(0, (int(e_counts.max()) - base + 127) // 128) + 8
    T_seq = np.array([4] * a + [3] * (W_PER_CORE - a), dtype=np.int64)
    caps = T_seq * 128
    C = int(caps.sum())
    win_off = np.concatenate([[0], np.cumsum(caps)])[:-1]

    cores = []
    unperm = np.empty(N_GRID, dtype=np.int64)
    for c in range(N_CORES):
        lo = c * N_SH
        mask = core_of_edge == c
        deg = np.bincount(dst[mask] - lo, minlength=N_SH)
        perm = _pack_core(deg, caps)
        inv = np.empty(N_SH, dtype=np.int64)
        inv[perm] = np.arange(N_SH)

        e_ids = np.nonzero(mask)[0]
        wslot = inv[dst[e_ids] - lo]
        w = wslot >> 7
        slot = wslot & 127
        order = np.lexsort((slot, w))
        e_ids, w, slot = e_ids[order], w[order], slot[order]
        cnt = np.bincount(w, minlength=W_PER_CORE)
        assert (cnt <= caps).all()
        within = np.arange(len(e_ids)) - np.repeat(
            np.concatenate([[0], np.cumsum(cnt)])[:-1], cnt
        )
        pos = win_off[w] + within

        dlf = np.full(C, -1.0, dtype=np.float32)
        dlf[pos] = slot.astype(np.float32)
        e0 = np.zeros((C, D), np.float32)
        e1 = np.zeros((C, D), np.float32)
        e2 = np.zeros((C, D), np.float32)
        e0[pos] = ef[e_ids]
        e1[pos] = mf[src[e_ids]]
        e2[pos] = gf[dst[e_ids]]

        gperm = perm + lo
        cores.append(
            dict(
                e0T=np.ascontiguousarray(e0.T),
                e1T=np.ascontiguousarray(e1.T),
                e2T=np.ascontiguousarray(e2.T),
                dlf=dlf,
                gridT=np.ascontiguousarray(gf[gperm].T),
                grid_res=np.ascontiguousarray(gf[gperm]),
            )
        )
        unperm[gperm] = c * N_SH + np.arange(N_SH)
    return T_seq, C, cores, unperm


# ------------------------------------------------------------- device program

def _build_program(T_seq, C, trivial_eln, trivial_nln, limit_windows=None):
    import concourse.bass as bass
    import concourse.tile as tile
    from concourse import bacc, mybir

    f32 = mybir.dt.float32
    f32r = mybir.dt.float32r
    AF = mybir.ActivationFunctionType
    OP = mybir.AluOpType

    nc = bacc.Bacc("TRN2", target_bir_lowering=False)

    e0T = nc.dram_tensor("e0T", [P, C], f32r, kind="ExternalInput")
    e1T = nc.dram_tensor("e1T", [P, C], f32r, kind="ExternalInput")
    e2T = nc.dram_tensor("e2T", [P, C], f32r, kind="ExternalInput")
    dlf = nc.dram_tensor("dlf", [C], f32, kind="ExternalInput")
    gridT = nc.dram_tensor("gridT", [P, N_SH], f32, kind="ExternalInput")
    grid_res = nc.dram_tensor("grid_res", [N_SH, D], f32, kind="ExternalInput")
    ew1 = nc.dram_tensor("ew1", [3 * D, HID], f32r, kind="ExternalInput")
    ew2 = nc.dram_tensor("ew2", [HID, D], f32, kind="ExternalInput")
    nw1 = nc.dram_tensor("nw1", [2 * D, HID], f32, kind="ExternalInput")
    nw2 = nc.dram_tensor("nw2", [HID, D], f32, kind="ExternalInput")
    eb1 = nc.dram_tensor("eb1", [HID], f32, kind="ExternalInput")
    nb1 = nc.dram_tensor("nb1", [HID], f32, kind="ExternalInput")
    eb2r = nc.dram_tensor("eb2r", [P, D], f32, kind="ExternalInput")
    nb2r = nc.dram_tensor("nb2r", [P, D], f32, kind="ExternalInput")
    iota = nc.dram_tensor("iota", [P, P], f32, kind="ExternalInput")
    # general-LN scale/shift (replicated rows); loaded only if nontrivial
    egr = nc.dram_tensor("egr", [P, D], f32, kind="ExternalInput")
    ebr = nc.dram_tensor("ebr", [P, D], f32, kind="ExternalInput")
    ngr = nc.dram_tensor("ngr", [P, D], f32, kind="ExternalInput")
    nbr = nc.dram_tensor("nbr", [P, D], f32, kind="ExternalInput")
    outp = nc.dram_tensor("outp", [N_SH, D], f32, kind="ExternalOutput")

    caps = [int(t) * 128 for t in T_seq]
    win_off = np.concatenate([[0], np.cumsum(caps)])[:-1]
    n_win = len(T_seq) if limit_windows is None else limit_windows

    with tile.TileContext(nc) as tc:
        with (
            tc.tile_pool(name="singles", bufs=1) as singles,
            tc.tile_pool(name="streams", bufs=3) as streams,
            tc.tile_pool(name="work", bufs=4) as work,
            tc.tile_pool(name="hbuf", bufs=6) as hbuf,
            tc.tile_pool(name="ph", bufs=3, space="PSUM") as ph,
            tc.tile_pool(name="pp", bufs=3, space="PSUM") as pp,
            tc.tile_pool(name="pagg", bufs=2, space="PSUM") as pagg,
        ):
            # ---- constants / weights
            w1s = singles.tile([P, 3, HID], f32r)
            nc.sync.dma_start(out=w1s, in_=ew1.rearrange("(c p) h -> p c h", p=P))
            w2s = singles.tile([P, 2, D], f32)
            nc.sync.dma_start(out=w2s, in_=ew2.rearrange("(c p) d -> p c d", p=P))
            nw1s = singles.tile([P, 2, HID], f32)
            nc.sync.dma_start(out=nw1s, in_=nw1.rearrange("(c p) h -> p c h", p=P))
            nw2s = singles.tile([P, 2, D], f32)
            nc.sync.dma_start(out=nw2s, in_=nw2.rearrange("(c p) d -> p c d", p=P))
            eb1s = singles.tile([P, 2], f32)
            nc.sync.dma_start(out=eb1s, in_=eb1.rearrange("(c p) -> p c", p=P))
            nb1s = singles.tile([P, 2], f32)
            nc.sync.dma_start(out=nb1s, in_=nb1.rearrange("(c p) -> p c", p=P))
            eb2s = singles.tile([P, D], f32)
            nc.sync.dma_start(out=eb2s, in_=eb2r[:])
            nb2s = singles.tile([P, D], f32)
            nc.sync.dma_start(out=nb2s, in_=nb2r[:])
            iotas = singles.tile([P, P], f32)
            nc.sync.dma_start(out=iotas, in_=iota[:])
            epss = singles.tile([P, 1], f32)
            nc.vector.memset(epss, EPS)
            egs = ebs = ngs = nbs = None
            if not trivial_eln:
                egs = singles.tile([P, D], f32)
                nc.sync.dma_start(out=egs, in_=egr[:])
                ebs = singles.tile([P, D], f32)
                nc.sync.dma_start(out=ebs, in_=ebr[:])
            if not trivial_nln:
                ngs = singles.tile([P, D], f32)
                nc.sync.dma_start(out=ngs, in_=ngr[:])
                nbs = singles.tile([P, D], f32)
                nc.sync.dma_start(out=nbs, in_=nbr[:])

            def layer_norm(x, g, b):
                # x: SBUF [128, D]; in-place LN along free dim
                st = work.tile([P, 6], f32, tag="st")
                nc.vector.bn_stats(st, x)
                mv = work.tile([P, 2], f32, tag="mv")
                nc.vector.bn_aggr(mv, st)
                rstd = work.tile([P, 1], f32, tag="rstd")
                nc.scalar.activation(out=rstd, in_=mv[:, 1:2], func=AF.Sqrt,
                                     bias=epss, scale=1.0)
                nc.vector.reciprocal(rstd, rstd)
                nc.vector.tensor_scalar(
                    out=x, in0=x, scalar1=mv[:, 0:1], scalar2=rstd,
                    op0=OP.subtract, op1=OP.mult)
                if g is not None:
                    nc.vector.tensor_tensor(out=x, in0=x, in1=g, op=OP.mult)
                    nc.vector.tensor_tensor(out=x, in0=x, in1=b, op=OP.add)

            aggb = None
            for w in range(n_win):
                T = int(T_seq[w])
                ET = T * 128
                off = int(win_off[w])
                sw_i = w % SW

                e0t = streams.tile([P, 512], f32r, tag="e0")
                nc.sync.dma_start(out=e0t[:, :ET], in_=e0T[:, off : off + ET])
                e1t = streams.tile([P, 512], f32r, tag="e1")
                nc.sync.dma_start(out=e1t[:, :ET], in_=e1T[:, off : off + ET])
                e2t = streams.tile([P, 512], f32r, tag="e2")
                nc.sync.dma_start(out=e2t[:, :ET], in_=e2T[:, off : off + ET])
                dlt = streams.tile([P, 4], f32, tag="dl")
                nc.sync.dma_start(
                    out=dlt[:, :T],
                    in_=dlf[off : off + ET].rearrange("(t p) -> p t", p=P))

                # edge L1: H[hc] [128h, ET]
                hts = []
                for hc in range(2):
                    hp = ph.tile([P, 512], f32, tag="h512")
                    for kc, srct in enumerate((e0t, e1t, e2t)):
                        nc.tensor.matmul(
                            hp[:, :ET],
                            lhsT=w1s[:, kc, hc * P : (hc + 1) * P],
                            rhs=srct[:, :ET],
                            start=(kc == 0), stop=(kc == 2))
                    hs = hbuf.tile([P, 512], f32, tag="hs")
                    nc.scalar.activation(out=hs[:, :ET], in_=hp[:, :ET],
                                         func=AF.Silu, bias=eb1s[:, hc : hc + 1])
                    hts.append(hs)

                aggp = pagg.tile([P, P], f32, tag="aggT")
                for t in range(T):
                    sl = slice(t * P, (t + 1) * P)
                    ef2p = pp.tile([P, P], f32, tag="p128")
                    for hc in range(2):
                        nc.tensor.matmul(
                            ef2p,
                            lhsT=hts[hc][:, sl],
                            rhs=w2s[:, hc, :],
                            start=(hc == 0), stop=(hc == 1))
                    ef2s = work.tile([P, D], f32, tag="ef2")
                    nc.vector.tensor_tensor(out=ef2s, in0=ef2p, in1=eb2s, op=OP.add)
                    layer_norm(ef2s, egs, ebs)
                    oh = work.tile([P, P], f32, tag="oh")
                    nc.vector.tensor_tensor(
                        out=oh, in0=dlt[:, t : t + 1].to_broadcast([P, P]),
                        in1=iotas, op=OP.is_equal)
                    nc.tensor.matmul(
                        aggp, lhsT=ef2s, rhs=oh,
                        start=(t == 0), stop=(t == T - 1))

                if sw_i == 0:
                    aggb = hbuf.tile([P, 512], f32, tag="aggb")
                nc.scalar.copy(out=aggb[:, sw_i * P : (sw_i + 1) * P], in_=aggp)

                # ---- node stage every SW windows
                if sw_i == SW - 1:
                    sw = w // SW
                    nsl = slice(sw * 512, (sw + 1) * 512)
                    gt = streams.tile([P, 512], f32, tag="gt")
                    nc.gpsimd.dma_start(out=gt, in_=gridT[:, nsl])
                    h2s = []
                    for hc in range(2):
                        h2p = ph.tile([P, 512], f32, tag="h512")
                        nc.tensor.matmul(
                            h2p, lhsT=nw1s[:, 0, hc * P : (hc + 1) * P],
                            rhs=aggb, start=True, stop=False)
                        nc.tensor.matmul(
                            h2p, lhsT=nw1s[:, 1, hc * P : (hc + 1) * P],
                            rhs=gt, start=False, stop=True)
                        h2 = hbuf.tile([P, 512], f32, tag="hs")
                        nc.scalar.activation(out=h2, in_=h2p, func=AF.Silu,
                                             bias=nb1s[:, hc : hc + 1])
                        h2s.append(h2)
                    for nt in range(4):
                        sl = slice(nt * P, (nt + 1) * P)
                        o2p = pp.tile([P, P], f32, tag="p128")
                        for hc in range(2):
                            nc.tensor.matmul(
                                o2p, lhsT=h2s[hc][:, sl],
                                rhs=nw2s[:, hc, :],
                                start=(hc == 0), stop=(hc == 1))
                        o2s = work.tile([P, D], f32, tag="o2")
                        nc.vector.tensor_tensor(out=o2s, in0=o2p, in1=nb2s, op=OP.add)
                        layer_norm(o2s, ngs, nbs)
                        rows = slice(sw * 512 + nt * P, sw * 512 + (nt + 1) * P)
                        gr = work.tile([P, D], f32, tag="gr")
                        nc.gpsimd.dma_start(out=gr, in_=grid_res[rows, :])
                        nc.vector.tensor_tensor(out=o2s, in0=o2s, in1=gr, op=OP.add)
                        nc.gpsimd.dma_start(out=outp[rows, :], in_=o2s)

    nc.finalize()
    return nc


# ----------------------------------------------------------------- entrypoint

def kernel(**inputs):
    import os

    from concourse.bass_utils import run_bass_kernel_spmd

    trace = bool(int(os.environ.get("KERNEL_TRACE", "0")))
    limit = os.environ.get("KERNEL_LIMIT_WINDOWS")
    limit = int(limit) if limit else None

    import time as _time
    _t0 = _time.time()
    T_seq, C, cores, unperm = _prepare(inputs)
    print(f"prep: {_time.time()-_t0:.1f}s", flush=True)

    eg = np.asarray(inputs["eg"], np.float32)
    ebeta = np.asarray(inputs["ebeta"], np.float32)
    ng = np.asarray(inputs["ng"], np.float32)
    nbeta = np.asarray(inputs["nbeta"], np.float32)
    trivial_eln = bool(np.all(eg == 1.0) and np.all(ebeta == 0.0))
    trivial_nln = bool(np.all(ng == 1.0) and np.all(nbeta == 0.0))

    _t0 = _time.time()
    nc = _build_program(T_seq, C, trivial_eln, trivial_nln,
                        limit_windows=limit)
    print(f"build: {_time.time()-_t0:.1f}s", flush=True)

    shared = dict(
        ew1=np.ascontiguousarray(inputs["eW1"], dtype=np.float32),
        ew2=np.ascontiguousarray(inputs["eW2"], dtype=np.float32),
        nw1=np.ascontiguousarray(inputs["nW1"], dtype=np.float32),
        nw2=np.ascontiguousarray(inputs["nW2"], dtype=np.float32),
        eb1=np.ascontiguousarray(inputs["eb1"], dtype=np.float32),
        nb1=np.ascontiguousarray(inputs["nb1"], dtype=np.float32),
        eb2r=np.ascontiguousarray(
            np.broadcast_to(np.asarray(inputs["eb2"], np.float32), (P, D))),
        nb2r=np.ascontiguousarray(
            np.broadcast_to(np.asarray(inputs["nb2"], np.float32), (P, D))),
        iota=np.ascontiguousarray(
            np.broadcast_to(np.arange(P, dtype=np.float32), (P, P))),
        egr=np.ascontiguousarray(np.broadcast_to(eg, (P, D))),
        ebr=np.ascontiguousarray(np.broadcast_to(ebeta, (P, D))),
        ngr=np.ascontiguousarray(np.broadcast_to(ng, (P, D))),
        nbr=np.ascontiguousarray(np.broadcast_to(nbeta, (P, D))),
    )
    in_maps = []
    for c in range(N_CORES):
        m = dict(shared)
        m.update(cores[c])
        in_maps.append(m)

    _t0 = _time.time()
    res = run_bass_kernel_spmd(nc, in_maps, core_ids=list(range(N_CORES)),
                               trace=trace)
    print(f"compile+exec: {_time.time()-_t0:.1f}s", flush=True)
    if res.exec_time_ns is not None:
        print(f"HW exec time: {res.exec_time_ns} ns", flush=True)
    full = np.concatenate([res.results[c]["outp"] for c in range(N_CORES)], axis=0)
    return np.ascontiguousarray(full[unperm])



# revision 69
# speedup vs baseline: 1.9663x; 1.9663x over previous
"""MeshGraphDecoder Trainium2 kernel (8-core SPMD), v2.

Sharding: grid nodes (and their incoming edges) are partitioned across 8
cores; mesh features and MLP weights are replicated (mesh/dst features
gathered on host into per-edge streams). Within a core, nodes are packed
into 256 windows of 128 nodes such that each window's incoming-edge
count fits a global per-window capacity schedule (T_w*128 slots, T_w in
{3,4}); the schedule is shared by all cores so one SPMD program serves
all 8.

Key design points (vs the v1 baseline, ~1.9x faster):
  - all matmuls and streams in bf16: PE does 1 cyc/row at any tile
    width (fp32 is 4x slower below 256 output columns), DMA halves.
  - LayerNorm mean is folded into W2 on the host (W2' = W2 - rowwise
    colmean) so the edge/node MLP outputs are exactly centered; the
    device only needs sum-of-squares -> rstd = rsqrt(ssq/D + eps),
    computed on DVE with a bit-trick seed + 1 Newton iteration (no
    ACT-table sqrt: Silu and Sqrt live in different activation tables
    and each switch costs a 1.3us table reload).
  - rstd is folded into the aggregation one-hot (oh' = (iota==slot) *
    rstd via one chained tensor_scalar per tile), so edge features are
    never renormalized explicitly; the node LN folds rstd2 and the
    grid residual into one scalar_tensor_tensor per output tile.
  - elementwise work is spread: ACT (SiLU, c PSUM->SBUF copy for even
    windows), DVE (grouped ssq reduce, newton, aggb copy, node out,
    odd-window copies), GPSIMD (squares, even-window one-hots).
  - three-deep supertile software pipeline (4 windows each): phase A
    (stream DMA, L1 matmuls one window ahead of L2, SiLU, LN stats)
    for supertile s runs while phase B (one-hot, aggregation) of s-1
    and the node stage of s-2 fill PE/vector gaps, so the cross-engine
    LN-stats chain never stalls the PE.
  - PSUM: H tiles [128,2,512]x2, c/o2 tiles [128,512]x2, agg
    [128,512]x2 = exactly 8 banks; all matmul accumulation groups are
    bank-aligned.

TimelineSim (the grading cost model): 548,030 ns/core; engine busy:
PE 462us (83%), ACT 446us, DVE 431us, GPSIMD ~340us, DMA 333us.
"""

import numpy as np

N_MESH = 40962
N_GRID = 262144
N_EDGE = 786432
D = 128
HID = 256
EPS = 1e-5
N_CORES = 8
N_SH = N_GRID // N_CORES
W_PER_CORE = N_SH // 128
P = 128
SW = 4  # windows per node-stage supertile


# ----------------------------------------------------------------- host prep

def _pack_core(degrees, caps):
    n = len(degrees)
    n_win = len(caps)
    caps = np.asarray(caps, dtype=np.int64)
    order = np.argsort(-degrees, kind="stable")
    sums = np.zeros(n_win, dtype=np.int64)
    cnts = np.zeros(n_win, dtype=np.int64)
    assign = np.empty(n, dtype=np.int64)
    mean = degrees.sum() / n
    caps_f = caps.astype(np.float64)
    for nid in order:
        slack = (caps_f - sums) - mean * (128 - cnts)
        slack[cnts >= 128] = -np.inf
        w = int(np.argmax(slack))
        assign[nid] = w
        sums[w] += degrees[nid]
        cnts[w] += 1
    members = [list(np.nonzero(assign == w)[0]) for w in range(n_win)]
    for _ in range(200000):
        over = np.nonzero(sums > caps)[0]
        if len(over) == 0:
            break
        w = int(over[0])
        mw = members[w]
        a = max(mw, key=lambda i: degrees[i])
        v = int(np.argmax(caps - sums))
        mv = members[v]
        b = min(mv, key=lambda i: degrees[i])
        da, db = int(degrees[a]), int(degrees[b])
        assert da > db and (caps[v] - sums[v]) >= (da - db), "repair stuck"
        mw.remove(a); mv.remove(b)
        mw.append(b); mv.append(a)
        sums[w] += db - da
        sums[v] += da - db
    else:
        raise RuntimeError("window repair did not converge")
    perm = np.empty(n, dtype=np.int64)
    for w in range(n_win):
        perm[w * 128 : (w + 1) * 128] = members[w]
    return perm


def _prepare(inputs):
    from concourse import mybir

    bf16 = mybir.dt.np(mybir.dt.bfloat16)

    dst = np.asarray(inputs["dst_idx"]).astype(np.int64)
    src = np.asarray(inputs["src_idx"]).astype(np.int64)
    ef = np.asarray(inputs["m2g_efeat"], dtype=np.float32)
    gf = np.asarray(inputs["grid_nfeat"], dtype=np.float32)
    mf = np.asarray(inputs["mesh_nfeat"], dtype=np.float32)

    core_of_edge = dst // N_SH
    e_counts = np.bincount(core_of_edge, minlength=N_CORES)
    base = W_PER_CORE * 3 * 128
    a = max(0, (int(e_counts.max()) - base + 127) // 128) + 8
    T_seq = np.array([4] * a + [3] * (W_PER_CORE - a), dtype=np.int64)
    caps = T_seq * 128
    C = int(caps.sum())
    win_off = np.concatenate([[0], np.cumsum(caps)])[:-1]
    cumT = np.concatenate([[0], np.cumsum(T_seq)])[:-1]
    sumT = int(T_seq.sum())

    ef16 = ef.astype(bf16)
    mf16 = mf.astype(bf16)
    gf16 = gf.astype(bf16)

    cores = []
    unperm = np.empty(N_GRID, dtype=np.int64)
    for c in range(N_CORES):
        lo = c * N_SH
        mask = core_of_edge == c
        deg = np.bincount(dst[mask] - lo, minlength=N_SH)
        perm = _pack_core(deg, caps)
        inv = np.empty(N_SH, dtype=np.int64)
        inv[perm] = np.arange(N_SH)

        e_ids = np.nonzero(mask)[0]
        wslot = inv[dst[e_ids] - lo]
        w = wslot >> 7
        slot = wslot & 127
        order = np.lexsort((slot, w))
        e_ids, w, slot = e_ids[order], w[order], slot[order]
        cnt = np.bincount(w, minlength=W_PER_CORE)
        assert (cnt <= caps).all()
        within = np.arange(len(e_ids)) - np.repeat(
            np.concatenate([[0], np.cumsum(cnt)])[:-1], cnt
        )
        pos = win_off[w] + within

        # packed transposed stream: per window [e0 | e1 | e2], each
        # [128, ET]; column offset of window w is 3*win_off[w]
        eall = np.zeros((P, 3 * C), dtype=bf16)
        col = (3 * win_off[w] + within).astype(np.int64)
        # e0 at block offset 0, e1 at +ET, e2 at +2*ET (ET = caps[w])
        eall[:, col] = ef16[e_ids].T
        eall[:, col + caps[w]] = mf16[src[e_ids]].T
        eall[:, col + 2 * caps[w]] = gf16[dst[e_ids]].T

        # slot table: dlall[p, cumT[w]+t] = slot of edge (t*128+p) of
        # window w; padded slots point at -1 (matches no node)
        dl = np.full((P, sumT), -1.0, dtype=np.float32)
        tt = within >> 7
        pp = within & 127
        dl[pp, cumT[w] + tt] = slot.astype(np.float32)

        gperm = perm + lo
        cores.append(
            dict(
                eall=eall,
                dlall=dl,
                gridT=np.ascontiguousarray(gf16[gperm].T),
                grid_res=np.ascontiguousarray(gf16[gperm]),
            )
        )
        unperm[gperm] = c * N_SH + np.arange(N_SH)
    return T_seq, C, cores, unperm


# ------------------------------------------------------------- device program

def _build_program(T_seq, C, trivial, limit_windows=None):
    import concourse.bass as bass
    import concourse.tile as tile
    from concourse import bacc, mybir

    f32 = mybir.dt.float32
    bf16 = mybir.dt.bfloat16
    AF = mybir.ActivationFunctionType
    OP = mybir.AluOpType

    nc = bacc.Bacc("TRN2", target_bir_lowering=False)

    sumT = int(T_seq.sum())
    eallT = nc.dram_tensor("eall", [P, 3 * C], bf16, kind="ExternalInput")
    dlall = nc.dram_tensor("dlall", [P, sumT], f32, kind="ExternalInput")
    gridT = nc.dram_tensor("gridT", [P, N_SH], bf16, kind="ExternalInput")
    grid_res = nc.dram_tensor("grid_res", [N_SH, D], bf16, kind="ExternalInput")
    ew1 = nc.dram_tensor("ew1", [3 * D, HID], bf16, kind="ExternalInput")
    ew2 = nc.dram_tensor("ew2", [HID, D], bf16, kind="ExternalInput")
    nw1 = nc.dram_tensor("nw1", [2 * D, HID], bf16, kind="ExternalInput")
    nw2 = nc.dram_tensor("nw2", [HID, D], bf16, kind="ExternalInput")
    iota = nc.dram_tensor("iota", [P, P], bf16, kind="ExternalInput")
    # general-path (non-trivial biases / LN affine) tensors
    eb1r = nc.dram_tensor("eb1r", [P, 2], f32, kind="ExternalInput")
    nb1r = nc.dram_tensor("nb1r", [P, 2], f32, kind="ExternalInput")
    eb2r = nc.dram_tensor("eb2r", [P, D], f32, kind="ExternalInput")
    egr = nc.dram_tensor("egr", [P, 1], f32, kind="ExternalInput")
    ngr = nc.dram_tensor("ngr", [P, D], f32, kind="ExternalInput")
    ebdeg = nc.dram_tensor("ebdeg", [2, N_SH], bf16, kind="ExternalInput")
    outp = nc.dram_tensor("outp", [N_SH, D], f32, kind="ExternalOutput")

    caps = [int(t) * 128 for t in T_seq]
    win_off = np.concatenate([[0], np.cumsum(caps)])[:-1]
    cumT = np.concatenate([[0], np.cumsum(T_seq)])[:-1]
    n_win = len(T_seq) if limit_windows is None else limit_windows

    with tile.TileContext(nc) as tc:
        with (
            tc.tile_pool(name="singles", bufs=1) as singles,
            tc.tile_pool(name="streams", bufs=3) as streams,
            tc.tile_pool(name="hbuf", bufs=4) as hbuf,
            tc.tile_pool(name="cbuf", bufs=8) as cbuf,
            tc.tile_pool(name="sqbuf", bufs=4) as sqbuf,
            tc.tile_pool(name="ohbuf", bufs=5) as ohbuf,
            tc.tile_pool(name="aggbuf", bufs=3) as aggbuf,
            tc.tile_pool(name="work", bufs=6) as work,
            tc.tile_pool(name="nodeb", bufs=3) as nodeb,
            tc.tile_pool(name="ph", bufs=4, space="PSUM") as ph,
            tc.tile_pool(name="pc", bufs=2, space="PSUM") as pc,
            tc.tile_pool(name="pagg", bufs=2, space="PSUM") as pagg,
        ):
            # ---- critical weights first (the first L1 waits on these)
            w1s = singles.tile([P, 3, HID], bf16)
            nc.sync.dma_start(out=w1s, in_=ew1.rearrange("(c p) h -> p c h", p=P))
            w2s = singles.tile([P, 2, D], bf16)
            nc.sync.dma_start(out=w2s, in_=ew2.rearrange("(c p) d -> p c d", p=P))
            eb1s = nb1s = eb2s = egs = ngs = ebdegs = None
            if not trivial:
                eb1s = singles.tile([P, 2], f32)
                nc.sync.dma_start(out=eb1s, in_=eb1r[:])
                nb1s = singles.tile([P, 2], f32)
                nc.sync.dma_start(out=nb1s, in_=nb1r[:])
                eb2s = singles.tile([P, D], f32)
                nc.sync.dma_start(out=eb2s, in_=eb2r[:])
                egs = singles.tile([P, 1], f32)
                nc.sync.dma_start(out=egs, in_=egr[:])
                ngs = singles.tile([P, D], f32)
                nc.sync.dma_start(out=ngs, in_=ngr[:])
                ebdegs = singles.tile([2, N_SH], bf16)
                nc.sync.dma_start(out=ebdegs, in_=ebdeg[:])

            # newton-rsqrt helper: rstd_out[:, :n] = 1/sqrt(v[:, :n])
            # (bit-trick seed + 1 newton iteration -> ~1.7e-3 max rel
            # err, well inside the LN tolerance; no ACT tables involved)
            MAGIC = 0x5f3759df
            i32 = mybir.dt.int32

            def newton_rsqrt(pool, tagp, v, n):
                ti = pool.tile([P, 20], i32, tag=tagp + "_i")
                nc.vector.tensor_scalar(
                    out=ti[:, :n], in0=v[:, :n].bitcast(i32), scalar1=1,
                    scalar2=None, op0=OP.logical_shift_right)
                nc.vector.tensor_scalar(
                    out=ti[:, :n], in0=ti[:, :n], scalar1=-1, scalar2=MAGIC,
                    op0=OP.mult, op1=OP.add)
                y = pool.tile([P, 20], f32, tag=tagp + "_y")
                t2 = pool.tile([P, 20], f32, tag=tagp + "_t")
                yf = ti.bitcast(f32)
                for it in range(1):
                    srcy = yf if it == 0 else y
                    nc.vector.tensor_tensor(
                        out=t2[:, :n], in0=srcy[:, :n], in1=srcy[:, :n], op=OP.mult)
                    nc.vector.tensor_tensor(
                        out=t2[:, :n], in0=t2[:, :n], in1=v[:, :n], op=OP.mult)
                    nc.vector.tensor_scalar(
                        out=t2[:, :n], in0=t2[:, :n], scalar1=-0.5, scalar2=1.5,
                        op0=OP.mult, op1=OP.add)
                    nc.vector.tensor_tensor(
                        out=y[:, :n], in0=srcy[:, :n], in1=t2[:, :n], op=OP.mult)
                return y

            n_st = (n_win + SW - 1) // SW

            def prefetch_stream(s):
                """issue the supertile's packed stream DMA well ahead."""
                if s >= n_st:
                    return None
                ws = list(range(s * SW, min((s + 1) * SW, n_win)))
                off0 = int(win_off[ws[0]])
                gcols = 3 * (int(win_off[ws[-1]]) + int(T_seq[ws[-1]]) * 128 - off0)
                est = streams.tile([P, 3 * 4 * 512], bf16, tag="eall")
                nc.sync.dma_start(
                    out=est[:, :gcols],
                    in_=eallT[:, 3 * off0 : 3 * off0 + gcols])
                return est

            def emit_A(s, est, prev_ctx=None, prev2_ctx=None):
                """edge MLP + LN stats for supertile s; returns phase-B ctx."""
                ws = list(range(s * SW, min((s + 1) * SW, n_win)))
                gT = sum(int(T_seq[w]) for w in ws)
                goff = int(cumT[ws[0]])
                ssqb = work.tile([P, 20], f32, tag="ssqb")
                csbs = {}
                off0 = int(win_off[ws[0]])

                def stage_l1(w):
                    T = int(T_seq[w])
                    ET = T * 128
                    off = int(win_off[w])
                    et = est[:, 3 * (off - off0) : 3 * (off - off0) + 3 * ET]
                    hs = hbuf.tile([P, 2, 512], bf16, tag="hs")
                    for hc in range(2):
                        hp = ph.tile([P, 512], f32, tag="hp")
                        for kc in range(3):
                            nc.tensor.matmul(
                                hp[:, :ET],
                                lhsT=w1s[:, kc, hc * P : (hc + 1) * P],
                                rhs=et[:, kc * ET : (kc + 1) * ET],
                                start=(kc == 0), stop=(kc == 2))
                        if trivial:
                            nc.scalar.activation(
                                out=hs[:, hc, :ET], in_=hp[:, :ET],
                                func=AF.Silu)
                        else:
                            nc.scalar.activation(
                                out=hs[:, hc, :ET], in_=hp[:, :ET],
                                func=AF.Silu, bias=eb1s[:, hc : hc + 1])
                    return hs

                def stage_l2(w, hs):
                    T = int(T_seq[w])
                    ET = T * 128
                    cp = pc.tile([P, 512], f32, tag="cp")
                    for t in range(T):
                        for hc in range(2):
                            nc.tensor.matmul(
                                cp[:, t * P : (t + 1) * P],
                                lhsT=hs[:, hc, t * P : (t + 1) * P],
                                rhs=w2s[:, hc, :],
                                start=(hc == 0), stop=(hc == 1))
                    if not trivial:
                        nc.vector.tensor_tensor(
                            out=cp[:, :ET].rearrange("p (t x) -> p t x", t=T),
                            in0=cp[:, :ET].rearrange("p (t x) -> p t x", t=T),
                            in1=eb2s.rearrange("p x -> p 1 x").to_broadcast([P, T, P]),
                            op=OP.add)
                    csb = cbuf.tile([P, 512], bf16, tag="csb")
                    if w % 2 == 0:
                        nc.scalar.copy(out=csb[:, :ET], in_=cp[:, :ET])
                    else:
                        nc.vector.tensor_copy(out=csb[:, :ET], in_=cp[:, :ET])
                    csbs[w] = csb
                    sq = sqbuf.tile([P, 512], bf16, tag="sq")
                    nc.gpsimd.tensor_mul(out=sq[:, :ET], in0=csb[:, :ET],
                                         in1=csb[:, :ET])
                    c0 = int(cumT[w]) - goff
                    T3 = sq[:, :ET].rearrange("p (t x) -> p t x", t=T)
                    nc.vector.tensor_reduce(
                        out=ssqb[:, c0 : c0 + T], in_=T3,
                        axis=mybir.AxisListType.X, op=OP.add)

                # L1 runs one window ahead of L2 (SiLU latency hides
                # behind the next window's L1 matmuls); the node L1 of
                # supertile s-2 covers the final window's SiLU latency
                hs_prev = None
                w_prev = None
                for i, w in enumerate(ws):
                    hs = stage_l1(w)
                    if hs_prev is not None:
                        stage_l2(w_prev, hs_prev)
                    hs_prev, w_prev = hs, w
                if prev2_ctx is not None:
                    emit_node_l1(prev2_ctx)
                stage_l2(w_prev, hs_prev)
                if prev_ctx is not None:
                    for i in range(len(prev_ctx["ws"])):
                        emit_agg_w(prev_ctx, i)
                    emit_agg_fin(prev_ctx)

                nc.vector.tensor_scalar(
                    out=ssqb[:, :gT], in0=ssqb[:, :gT], scalar1=1.0 / D,
                    scalar2=EPS, op0=OP.mult, op1=OP.add)
                rstds = newton_rsqrt(work, "er", ssqb, gT)
                return dict(ws=ws, goff=goff, csbs=csbs, rstds=rstds, s=s)

            def emit_oh(ctx):
                """one-hot * rstd tiles for a previous supertile (DVE-early)."""
                ohs = {}
                for w in ctx["ws"]:
                    T = int(T_seq[w])
                    toff = int(cumT[w])
                    c0 = toff - ctx["goff"]
                    oh = ohbuf.tile([P, 512], bf16, tag="oh")
                    for t in range(T):
                        eng = nc.gpsimd if w % 2 == 1 else nc.vector
                        eng.tensor_scalar(
                            out=oh[:, t * P : (t + 1) * P],
                            in0=iotas,
                            scalar1=dls[:, toff + t : toff + t + 1],
                            scalar2=ctx["rstds"][:, c0 + t : c0 + t + 1],
                            op0=OP.is_equal, op1=OP.mult)
                    ohs[w] = oh
                ctx["ohs"] = ohs

            def emit_agg_w(ctx, i):
                if i >= len(ctx["ws"]):
                    return
                if "aggp" not in ctx:
                    aggp = pagg.tile([P, 512], f32, tag="aggp")
                    ctx["aggp"] = aggp
                aggp = ctx["aggp"]
                w = ctx["ws"][i]
                T = int(T_seq[w])
                sw_i = w % SW
                csb = ctx["csbs"][w]
                oh = ctx["ohs"][w]
                for t in range(T):
                    nc.tensor.matmul(
                        aggp[:, sw_i * P : (sw_i + 1) * P],
                        lhsT=csb[:, t * P : (t + 1) * P],
                        rhs=oh[:, t * P : (t + 1) * P],
                        start=(t == 0), stop=(t == T - 1))

            def emit_agg_fin(ctx):
                aggb = aggbuf.tile([P, 512], bf16, tag="aggb")
                aggp = ctx["aggp"]
                nu = len(ctx["ws"]) * P
                if trivial:
                    nc.vector.tensor_copy(
                        out=aggb[:, :nu], in_=aggp[:, :nu])
                else:
                    nc.vector.tensor_scalar(
                        out=aggb[:, :nu], in0=aggp[:, :nu],
                        scalar1=egs[:, 0:1], scalar2=None, op0=OP.mult)
                ctx["aggb"] = aggb

            def emit_node_dma(ctx):
                if len(ctx["ws"]) < SW:
                    return
                sw = ctx["s"]
                nsl = slice(sw * 512, (sw + 1) * 512)
                gt = nodeb.tile([P, 512], bf16, tag="gt")
                nc.sync.dma_start(out=gt, in_=gridT[:, nsl])
                ctx["gt"] = gt
                gr = nodeb.tile([P, 4, P], bf16, tag="gr")
                nc.sync.dma_start(
                    out=gr,
                    in_=grid_res[nsl, :].rearrange("(t p) d -> p t d", p=P))
                ctx["gr"] = gr

            def emit_node_l1(ctx):
                if len(ctx["ws"]) < SW:
                    return
                sw = ctx["s"]
                aggb = ctx["aggb"]
                nsl = slice(sw * 512, (sw + 1) * 512)
                gt = ctx["gt"]
                h2s = nodeb.tile([P, 1024], bf16, tag="h2s")
                for hc in range(2):
                    h2p = ph.tile([P, 512], f32, tag="hp")
                    nc.tensor.matmul(
                        h2p,
                        lhsT=nw1s[:, 0, hc * P : (hc + 1) * P],
                        rhs=aggb, start=True, stop=(trivial))
                    if not trivial:
                        nc.tensor.matmul(
                            h2p,
                            lhsT=ebdegs[0:1, hc * P : (hc + 1) * P],
                            rhs=ebdegs[1:2, nsl],
                            start=False, stop=True)
                    nc.tensor.matmul(
                        h2p,
                        lhsT=nw1s[:, 1, hc * P : (hc + 1) * P],
                        rhs=gt, start=False, stop=True)
                    if trivial:
                        nc.scalar.activation(
                            out=h2s[:, hc * 512 : (hc + 1) * 512],
                            in_=h2p, func=AF.Silu)
                    else:
                        nc.scalar.activation(
                            out=h2s[:, hc * 512 : (hc + 1) * 512],
                            in_=h2p,
                            func=AF.Silu, bias=nb1s[:, hc : hc + 1])
                ctx["h2s"] = h2s

            def emit_node_l2(ctx):
                if len(ctx["ws"]) < SW:
                    return
                sw = ctx["s"]
                h2s = ctx["h2s"]
                nsl = slice(sw * 512, (sw + 1) * 512)
                o2p = pc.tile([P, 512], f32, tag="cp")
                for nt in range(4):
                    for hc in range(2):
                        nc.tensor.matmul(
                            o2p[:, nt * P : (nt + 1) * P],
                            lhsT=h2s[:, hc * 512 + nt * P : hc * 512 + (nt + 1) * P],
                            rhs=nw2s[:, hc, :],
                            start=(hc == 0), stop=(hc == 1))
                sq2 = sqbuf.tile([P, 512], bf16, tag="sq")
                nc.scalar.activation(out=sq2, in_=o2p, func=AF.Square)
                st2 = work.tile([P, 20], f32, tag="st2")
                nc.vector.tensor_reduce(
                    out=st2[:, :4],
                    in_=sq2.rearrange("p (t x) -> p t x", t=4),
                    axis=mybir.AxisListType.X, op=OP.add)
                nc.vector.tensor_scalar(
                    out=st2[:, :4], in0=st2[:, :4], scalar1=1.0 / D,
                    scalar2=EPS, op0=OP.mult, op1=OP.add)
                rstd2 = newton_rsqrt(work, "nr", st2, 4)
                gr = ctx["gr"]
                o2s = nodeb.tile([P, 4, P], f32, tag="o2s")
                for nt in range(4):
                    if trivial:
                        nc.vector.scalar_tensor_tensor(
                            out=o2s[:, nt, :],
                            in0=o2p[:, nt * P : (nt + 1) * P],
                            scalar=rstd2[:, nt : nt + 1],
                            in1=gr[:, nt, :],
                            op0=OP.mult, op1=OP.add)
                    else:
                        tmp = work.tile([P, P], f32, tag="tmp")
                        nc.vector.tensor_scalar(
                            out=tmp, in0=o2p[:, nt * P : (nt + 1) * P],
                            scalar1=rstd2[:, nt : nt + 1], scalar2=None,
                            op0=OP.mult)
                        nc.vector.tensor_tensor(
                            out=tmp, in0=tmp, in1=ngs, op=OP.mult)
                        nc.vector.tensor_tensor(
                            out=o2s[:, nt, :],
                            in0=tmp, in1=gr[:, nt, :], op=OP.add)
                half = slice(sw * 512, sw * 512 + 256)
                nc.sync.dma_start(
                    out=outp[half, :].rearrange("(t p) d -> p t d", p=P),
                    in_=o2s[:, 0:2, :])
                half2 = slice(sw * 512 + 256, (sw + 1) * 512)
                nc.sync.dma_start(
                    out=outp[half2, :].rearrange("(t p) d -> p t d", p=P),
                    in_=o2s[:, 2:4, :])

            # software pipeline: phase B (+node) of supertile s-1 overlaps
            # phase A of supertile s, so the cross-engine LN-stats chain of
            # s never stalls the PE at s-1's aggregation.
            prev = None
            prev2 = None
            ests = {0: prefetch_stream(0), 1: prefetch_stream(1)}
            # ---- non-critical singles after the first stream prefetches
            nw1s = singles.tile([P, 2, HID], bf16)
            nc.sync.dma_start(out=nw1s, in_=nw1.rearrange("(c p) h -> p c h", p=P))
            nw2s = singles.tile([P, 2, D], bf16)
            nc.sync.dma_start(out=nw2s, in_=nw2.rearrange("(c p) d -> p c d", p=P))
            iotas = singles.tile([P, P], bf16)
            nc.sync.dma_start(out=iotas, in_=iota[:])
            dls = singles.tile([P, sumT], f32)
            nc.sync.dma_start(out=dls, in_=dlall[:])
            epss = singles.tile([P, 1], f32)
            nc.vector.memset(epss, EPS)
            for s in range(n_st):
                if prev is not None:
                    emit_oh(prev)
                    emit_node_dma(prev)
                ests[s + 2] = prefetch_stream(s + 2)
                ctx = emit_A(s, ests.pop(s), prev_ctx=prev, prev2_ctx=prev2)
                if prev2 is not None:
                    emit_node_l2(prev2)
                prev2 = prev
                prev = ctx
            if prev2 is not None:
                emit_node_l1(prev2)
                emit_node_l2(prev2)
            if prev is not None:
                emit_oh(prev)
                emit_node_dma(prev)
                for i in range(len(prev["ws"])):
                    emit_agg_w(prev, i)
                emit_agg_fin(prev)
                emit_node_l1(prev)
                emit_node_l2(prev)

    nc.finalize()
    return nc


# ----------------------------------------------------------------- entrypoint

def kernel(**inputs):
    import os

    from concourse import mybir
    from concourse.bass_utils import run_bass_kernel_spmd

    bf16 = mybir.dt.np(mybir.dt.bfloat16)

    trace = bool(int(os.environ.get("KERNEL_TRACE", "0")))
    limit = os.environ.get("KERNEL_LIMIT_WINDOWS")
    limit = int(limit) if limit else None

    import time as _time
    _t0 = _time.time()
    T_seq, C, cores, unperm = _prepare(inputs)
    print(f"prep: {_time.time()-_t0:.1f}s", flush=True)

    eg = np.asarray(inputs["eg"], np.float32)
    ebeta = np.asarray(inputs["ebeta"], np.float32)
    ng = np.asarray(inputs["ng"], np.float32)
    nbeta = np.asarray(inputs["nbeta"], np.float32)
    eb1 = np.asarray(inputs["eb1"], np.float32)
    nb1 = np.asarray(inputs["nb1"], np.float32)
    eb2 = np.asarray(inputs["eb2"], np.float32)
    nb2 = np.asarray(inputs["nb2"], np.float32)
    trivial = bool(
        np.all(eg == 1.0) and np.all(ebeta == 0.0)
        and np.all(ng == 1.0) and np.all(nbeta == 0.0)
        and np.all(eb1 == 0.0) and np.all(nb1 == 0.0)
        and np.all(eb2 == 0.0) and np.all(nb2 == 0.0)
    )

    _t0 = _time.time()
    nc = _build_program(T_seq, C, trivial, limit_windows=limit)
    print(f"build: {_time.time()-_t0:.1f}s", flush=True)

    # centered second-layer weights: LN(x W2 + b2) with mean folded in
    eW2 = np.asarray(inputs["eW2"], np.float32)
    nW2 = np.asarray(inputs["nW2"], np.float32)
    eW2c = eW2 - eW2.mean(axis=1, keepdims=True)
    nW2c = nW2 - nW2.mean(axis=1, keepdims=True)
    eb2c = eb2 - eb2.mean()
    nb2c = nb2 - nb2.mean()

    nW1 = np.asarray(inputs["nW1"], np.float32)
    # general path: beta term of edge LN contributes (nW1_agg^T ebeta)
    # outer deg to the node hidden; grid_res carries nbeta
    bvec = (nW1[:D].T @ ebeta).astype(np.float32)  # [HID]

    shared = dict(
        ew1=np.ascontiguousarray(np.asarray(inputs["eW1"], np.float32)).astype(bf16),
        ew2=np.ascontiguousarray(eW2c).astype(bf16),
        nw1=np.ascontiguousarray(nW1).astype(bf16),
        nw2=np.ascontiguousarray(nW2c).astype(bf16),
        iota=np.ascontiguousarray(
            np.broadcast_to(np.arange(P, dtype=np.float32), (P, P))).astype(bf16),
        eb1r=np.ascontiguousarray(eb1.reshape(2, P).T),
        nb1r=np.ascontiguousarray(nb1.reshape(2, P).T),
        eb2r=np.ascontiguousarray(np.broadcast_to(eb2c, (P, D))),
        egr=np.ascontiguousarray(eg.reshape(P, 1)),
        ngr=np.ascontiguousarray(np.broadcast_to(ng, (P, D))),
    )
    in_maps = []
    dst = np.asarray(inputs["dst_idx"]).astype(np.int64)
    for c in range(N_CORES):
        m = dict(shared)
        cd = cores[c]
        # per-core degree table for the general-path beta fold
        lo = c * N_SH
        deg = np.bincount(dst[(dst // N_SH) == c] - lo, minlength=N_SH)
        gperm_inv = unperm[lo : lo + N_SH] - c * N_SH  # permuted pos of node
        degp = np.zeros(N_SH, np.float32)
        degp[gperm_inv] = deg
        ebdeg = np.zeros((2, N_SH), np.float32)
        ebdeg[0, :HID // 2] = 0  # row0 cols 0..255 hold bvec (padded)
        ebdeg[0, :HID] = bvec
        ebdeg[1] = degp
        m["ebdeg"] = np.ascontiguousarray(ebdeg).astype(bf16)
        if not np.all(np.asarray(inputs["nbeta"]) == 0.0):
            cd = dict(cd)
            cd["grid_res"] = (
                cd["grid_res"].astype(np.float32)
                + np.asarray(inputs["nbeta"], np.float32)
            ).astype(bf16)
        m.update(cd)
        in_maps.append(m)

    _t0 = _time.time()
    res = run_bass_kernel_spmd(nc, in_maps, core_ids=list(range(N_CORES)),
                               trace=trace)
    print(f"compile+exec: {_time.time()-_t0:.1f}s", flush=True)
    if res.exec_time_ns is not None:
        print(f"HW exec time: {res.exec_time_ns} ns", flush=True)
    else:
        try:
            from concourse.timeline_sim import TimelineSim

            t_ns = TimelineSim(nc).simulate()
            print(f"HW exec time: {int(t_ns)} ns "
                  f"(TimelineSim cost model; NTFF n/a)", flush=True)
        except Exception as e:
            print(f"TimelineSim estimate unavailable: {e}", flush=True)
    full = np.concatenate([res.results[c]["outp"] for c in range(N_CORES)], axis=0)
    return np.ascontiguousarray(full[unperm])
